# revision 32
# baseline (speedup 1.0000x reference)
# Causal multi-head self-attention (d_model=1024, 16 heads, s=2048, b=2) on
# 8 TRN2 NeuronCores. Sharding: batch (2) x head-groups (4 heads/core).
#
# Per-core dataflow (bf16 operands, fp32 PSUM accumulation):
#   - Host passes x[b] transposed ([D, S]) plus per-core weight slices laid
#     out so every matmul contraction lands on SBUF partitions:
#       wqT/wkT: [D, 256] with output dims permuted to (even | odd) halves so
#                RoPE becomes full-width ops (chunk0 = x1 rows, chunk1 = x2)
#       wvT:     [D, 256] natural;  woT: [256, D] rows for this head group
#       cosT/sinT: [128, S] rope tables (row p <-> pair index p % 32)
#     All inputs bf16 (rel err ~5e-3 vs the fp32 reference, far under the
#     2e-2 gate) which halves DMA bytes and SBUF footprint.
#   - QT/KT projections -> PSUM -> RoPE (DVE mults, gpsimd sub/add) -> SBUF
#   - V projection -> SBUF with a ones-column appended per (head, key-chunk)
#     so the PV matmul (M=65) also produces the softmax denominator row.
#   - Rotated Q/K are repacked head-contiguous (SBUF->SBUF DMA partition
#     permute) so each score block S^T[k, q] is a single K=64 bf16 matmul.
#   - exp on ScalarE straight out of PSUM (no max-subtraction: |scores/8| is
#     bounded ~12, exp stays finite), causal masking via column-windowed
#     matmuls/exp + one affine_select per diagonal block.
#   - PV accumulation in PSUM over key tiles; denominator row staged out of
#     PSUM fast (frees the bank), then reciprocal/broadcast/multiply.
#   - Output projection y = attnT.T @ woT -> DMA out [S, D] bf16; host
#     upcasts and sums the 4 partial y per batch.
#
# Emission is unit-granular: projection and y-projection matmul units are
# interleaved between attention key-blocks so the tensor engine never
# starves behind the ScalarE exp (~910ns/block vs ~850ns PE work/block).
#
# Hardware lessons baked in (CoreSim's cost model misses these):
#   - gpsimd (Pool/Q7) ops are far slower on HW than modeled and cannot
#     read PSUM at all; keep the normalize chain on DVE, unchunked.
#   - Per-DMA queue-issue overhead (~650ns HWDGE) argues for merged strided
#     DMAs in sim, but plain 2D per-chunk DMAs measure faster on HW
#     (XT_SPLIT/W_SPLIT default True).
#   - Measure with repeat-amplified A/B (the axon tunnel adds ~80ms fixed
#     RPC per execute; single-execute wall time says nothing).

import math

import ml_dtypes
import numpy as np

BF16 = ml_dtypes.bfloat16

B = 2
S = 2048
D = 1024
H = 16
HPC = 4  # heads per core
DK = 64
NCORES = 8
QSPAN = 512
KT = 128
SCALE = 1.0 / math.sqrt(DK)
ROPE_THETA = 10000.0

_PROGRAM_CACHE = {}

# pool-depth tunables (per-partition SBUF budget is ~192KB; keep total under)
BUFS_XT = 2
BUFS_ROTSP = 10
BUFS_E = 3
BUFS_Y = 3
BUFS_S2 = 2
BUFS_QK = 2
BUFS_VOT = 2


def _build_program(repeat=1, k64=True):
    import concourse.bacc as bacc
    import concourse.mybir as mybir
    import concourse.tile as tile

    F32 = mybir.dt.float32
    F32R = mybir.dt.float32r
    BF16 = mybir.dt.bfloat16
    ROT_BF16 = globals().get("_ROT_BF16", True)
    ROTDT = BF16 if ROT_BF16 else F32R
    ROTK_DT = ROTDT
    EXP = mybir.ActivationFunctionType.Exp

    def r(ap):
        return ap.bitcast(F32R)

    XT_SPLIT = globals().get("_XT_SPLIT", True)
    W_SPLIT = globals().get("_W_SPLIT", True)
    MM_BF16 = globals().get("_MM_BF16", True)
    MMDT = BF16 if MM_BF16 else F32R
    NORM_CHUNKED = globals().get("_NORM_CHUNKED", False)
    MASK_MUL = globals().get("_MASK_MUL", False)
    ROPE_DVE = globals().get("_ROPE_DVE", False)
    BE = BUFS_E if MM_BF16 else 2
    BY = BUFS_Y if MM_BF16 else 2
    BRS = BUFS_ROTSP if MM_BF16 else 8
    DRDT = BF16 if MM_BF16 else F32
    def mr(ap):
        return ap if MM_BF16 else ap.bitcast(F32R)

    def rr(ap):
        return ap if ROT_BF16 else ap.bitcast(F32R)
    nc = bacc.Bacc("TRN2", target_bir_lowering=False, debug=False, num_devices=NCORES)

    xT = nc.dram_tensor("xT", [D, S], DRDT, kind="ExternalInput").ap()
    wqT = nc.dram_tensor("wqT", [D, 2 * HPC * 32], DRDT, kind="ExternalInput").ap()
    wkT = nc.dram_tensor("wkT", [D, 2 * HPC * 32], DRDT, kind="ExternalInput").ap()
    wvT = nc.dram_tensor("wvT", [D, HPC * DK], DRDT, kind="ExternalInput").ap()
    woT = nc.dram_tensor("woT", [HPC * DK, D], DRDT, kind="ExternalInput").ap()
    cosT = nc.dram_tensor("cosT", [128, S], DRDT, kind="ExternalInput").ap()
    sinT = nc.dram_tensor("sinT", [128, S], DRDT, kind="ExternalInput").ap()
    y = nc.dram_tensor("y", [S, D], DRDT, kind="ExternalOutput").ap()

    NQS = S // QSPAN  # 4 q spans
    NKC = D // 128  # 8 contraction chunks for projections
    NSC = S // KT  # 16 key/seq chunks
    VW = DK + 1  # 65: V columns + ones column

    with tile.TileContext(nc) as tc:
        with (
            tc.tile_pool(name="persist", bufs=1) as persist,
            tc.tile_pool(name="wpool", bufs=1) as wpool,
            tc.tile_pool(name="stream", bufs=1) as stream,
            tc.tile_pool(name="rtmp", bufs=2) as rtmp,
            tc.tile_pool(name="epool", bufs=BE) as epool,
            tc.tile_pool(name="ypool", bufs=BY) as ypool,
            tc.tile_pool(name="psum", bufs=1, space="PSUM") as psum,
        ):
            # ---- persistent SBUF tensors ----
            cos_sb = persist.tile([128, S], DRDT, name="cos_sb")
            sin_sb = persist.tile([128, S], DRDT, name="sin_sb")
            if k64:
                # head-contiguous: cols [pair*S]; rows 64*hh..+64 per head
                rotqh = persist.tile([128, 2 * S], ROTDT, name="rotqh")
                rotkh = persist.tile([128, 2 * S], ROTK_DT, name="rotkh")
                rotq = rotk = None
            else:
                rotq = persist.tile([128, 2 * S], F32R, name="rotq")
                rotk = persist.tile([128, 2 * S], F32R, name="rotk")
            vsb = persist.tile([128, HPC * NSC * VW], MMDT, name="vsb")
            attn0 = persist.tile([128, S], MMDT, name="attn0")  # heads 0,1 dims
            attn1 = persist.tile([128, S], MMDT, name="attn1")  # heads 2,3 dims
            wo_sb = persist.tile([128, 2 * D], MMDT, name="wo_sb")

            # ---- weights in ----
            wq_sb = wpool.tile([128, NKC * 256], MMDT, name="wq_sb")
            wk_sb = wpool.tile([128, NKC * 256], MMDT, name="wk_sb")
            wv_sb = wpool.tile([128, NKC * 256], MMDT, name="wv_sb")
            if W_SPLIT:
                for kc in range(NKC):
                    sl = slice(128 * kc, 128 * kc + 128)
                    nc.scalar.dma_start(wq_sb[:, 256 * kc : 256 * kc + 256], mr(wqT[sl, :]))
                    nc.scalar.dma_start(wk_sb[:, 256 * kc : 256 * kc + 256], mr(wkT[sl, :]))
                    nc.scalar.dma_start(wv_sb[:, 256 * kc : 256 * kc + 256], mr(wvT[sl, :]))
            else:
                # merged weight loads: 2 DMAs per tensor (4 kc-chunks each) so
                # the scalar queue isn't serialized by 24 per-chunk issues
                for half in range(2):
                    cs, rs = 1024 * half, 512 * half
                    for w_sb, wT in ((wq_sb, wqT), (wk_sb, wkT), (wv_sb, wvT)):
                        nc.scalar.dma_start(
                            w_sb[:, cs : cs + 1024].rearrange("p (kc n) -> p kc n", kc=4),
                            mr(wT[rs : rs + 512, :]).rearrange("(kc p) n -> p kc n", kc=4),
                        )
            for p in range(2):
                nc.scalar.dma_start(
                    wo_sb[:, D * p : D * p + D], mr(woT[128 * p : 128 * p + 128, :])
                )
            nc.scalar.dma_start(cos_sb[:], cosT[:])
            nc.scalar.dma_start(sin_sb[:], sinT[:])

            if MASK_MUL:
                # persistent [128, KT] causal mask (1 where key p <= query c):
                # built once, then diagonal blocks mask via a DVE multiply
                # instead of a per-block gpsimd affine_select
                cmask_f = wpool.tile([128, KT], F32, name="cmask_f")
                cmask = wpool.tile([128, KT], MMDT, name="cmask")
                nc.vector.memset(cmask_f[:], 1.0)
                nc.gpsimd.affine_select(
                    out=cmask_f[:],
                    in_=cmask_f[:],
                    compare_op=mybir.AluOpType.is_ge,
                    fill=0.0,
                    base=0,
                    pattern=[[1, KT]],
                    channel_multiplier=-1,
                )
                nc.vector.tensor_copy(cmask[:], cmask_f[:])

            # ones columns of vsb: vsb[:, h*(NSC*VW) + c*VW + DK] = 1.0
            # (memset can't write f32r; stage in f32 and broadcast-copy)
            ones_src = wpool.tile([128, 1], F32, name="ones_src")
            nc.vector.memset(ones_src[:], 1.0)
            ones_ap = vsb.rearrange("p (h c w) -> p h c w", h=HPC, c=NSC)[
                :, :, :, DK : DK + 1
            ]
            nc.vector.tensor_copy(
                ones_ap, ones_src[:].unsqueeze(1).broadcast_to([128, HPC, NSC, 1])
            )


            # ---------------- phase builders (unit-granular emission) ----------------
            # Each phase is broken into small "units" (thunks emitting a few
            # instructions). _emit_once interleaves proj/yproj units between
            # attention key-blocks so PE never starves behind the ScalarE exp
            # (exp is ~910ns/block vs ~850ns of PE work per block).
            pending_perm = {}  # (j, widx) -> [rsp1, rsp2, flushed_pairs]

            def flush_perm(j_, pair):
                qs_ = QSPAN * j_
                for widx in range(2):
                    ent = pending_perm.get((j_, widx))
                    if ent is None or pair in ent[2]:
                        continue
                    ent[2].add(pair)
                    roth = rotqh if widx == 0 else rotkh
                    for c, rc in ((0, ent[0]), (1, ent[1])):
                        for hh in range(2):
                            h = 2 * pair + hh
                            nc.sync.dma_start(
                                roth[
                                    64 * hh + 32 * c : 64 * hh + 32 * c + 32,
                                    pair * S + qs_ : pair * S + qs_ + QSPAN,
                                ],
                                rc[32 * h : 32 * h + 32, :],
                            )
                    if len(ent[2]) == 2:
                        del pending_perm[(j_, widx)]

            def proj_units(j):
                """[setup+qk0, v0, v1, qk1, v2, v3] units for span j."""
                qs = QSPAN * j
                st = {}

                def setup():
                    xt_t = stream.tile(
                        [128, NKC * QSPAN], MMDT, tag="xt", bufs=BUFS_XT,
                        name=f"xt_{j}",
                    )
                    # span 0 is latency-critical: split its load so the first
                    # projection matmuls can start after the first half lands
                    nhalf = NKC if XT_SPLIT else (2 if j == 0 else 1)
                    hk = NKC // nhalf
                    for hf in range(nhalf):
                        s_dst = xt_t[:, QSPAN * hk * hf : QSPAN * hk * (hf + 1)]
                        s_src = mr(xT[128 * hk * hf : 128 * hk * (hf + 1), qs : qs + QSPAN])
                        if hk > 1:
                            s_dst = s_dst.rearrange("p (kc s) -> p kc s", kc=hk)
                            s_src = s_src.rearrange("(kc p) s -> p kc s", kc=hk)
                        nc.sync.dma_start(s_dst, s_src)
                    st["xts"] = [
                        xt_t[:, QSPAN * kc : QSPAN * kc + QSPAN] for kc in range(NKC)
                    ]

                def qk_proj(widx):
                    xts = st["xts"]
                    w_sb = (wq_sb, wk_sb)[widx]
                    pss = []
                    for c in range(2):
                        ps = psum.tile(
                            [128, QSPAN], F32, tag="qk", bufs=BUFS_QK,
                            name=f"psqk_{j}_{c}",
                        )
                        for kc in range(NKC):
                            nc.tensor.matmul(
                                ps[:],
                                mr(w_sb[:, 256 * kc + 128 * c : 256 * kc + 128 * c + 128]),
                                mr(xts[kc]),
                                start=(kc == 0),
                                stop=(kc == NKC - 1),
                            )
                        pss.append(ps)
                    t1 = rtmp.tile([128, QSPAN], F32, tag="t1", name=f"t1_{j}")
                    t2 = rtmp.tile([128, QSPAN], F32, tag="t2", name=f"t2_{j}")
                    t3 = rtmp.tile([128, QSPAN], F32, tag="t3", name=f"t3_{j}")
                    t4 = rtmp.tile([128, QSPAN], F32, tag="t4", name=f"t4_{j}")
                    cos_t = cos_sb[:, qs : qs + QSPAN]
                    sin_t = sin_sb[:, qs : qs + QSPAN]
                    # read pss[0] with both ops first so its PSUM slot frees
                    # earlier for the next projection's matmuls
                    nc.vector.tensor_mul(t1[:], pss[0][:], cos_t)
                    nc.vector.tensor_mul(t3[:], pss[0][:], sin_t)
                    nc.vector.tensor_mul(t2[:], pss[1][:], sin_t)
                    nc.vector.tensor_mul(t4[:], pss[1][:], cos_t)
                    rdt = ROTDT if widx == 0 else ROTK_DT
                    rsp1 = stream.tile(
                        [128, QSPAN], rdt, tag="rotsp", bufs=BRS,
                        name=f"rsp_{j}_{widx}_a",
                    )
                    rsp2 = stream.tile(
                        [128, QSPAN], rdt, tag="rotsp", bufs=BRS,
                        name=f"rsp_{j}_{widx}_b",
                    )
                    rope_eng = nc.vector if ROPE_DVE else nc.gpsimd
                    rope_eng.tensor_sub(rsp1[:], t1[:], t2[:])
                    rope_eng.tensor_add(rsp2[:], t3[:], t4[:])
                    # defer the head-permute DMAs until the attention span that
                    # consumes them (per pair) so they don't clog the sync queue
                    pending_perm[(j, widx)] = [rsp1, rsp2, set()]

                def v_proj(scl):
                    xts = st["xts"]
                    sc = (QSPAN // KT) * j + scl
                    psv = psum.tile(
                        [128, HPC * DK], F32, tag="vot", bufs=BUFS_VOT,
                        name=f"psv_{sc}",
                    )
                    for kc in range(NKC):
                        nc.tensor.matmul(
                            psv[:],
                            mr(xts[kc][:, KT * scl : KT * scl + KT]),
                            mr(wv_sb[:, 256 * kc : 256 * kc + 256]),
                            start=(kc == 0),
                            stop=(kc == NKC - 1),
                        )
                    dst = vsb.rearrange("p (h c w) -> p h c w", h=HPC, c=NSC)[
                        :, :, sc, 0:DK
                    ]
                    srcv = psv.rearrange("p (h d) -> p h d", h=HPC)
                    nc.vector.tensor_copy(dst, srcv)

                def u0():
                    setup()
                    qk_proj(0)

                return [
                    u0,
                    lambda: v_proj(0),
                    lambda: v_proj(1),
                    lambda: qk_proj(1),
                    lambda: v_proj(2),
                    lambda: v_proj(3),
                ]

            def attn_pair_units(pair, j):
                """[flush+block0, block1.., norm_hh0, norm_hh1] for one pair."""
                attn_t = attn0 if pair == 0 else attn1
                qs = QSPAN * j
                nkt = (QSPAN // KT) * j + (QSPAN // KT)
                st = {}

                def block(kt_i):
                    if kt_i == 0:
                        flush_perm(j, pair)
                        st["ots"] = [
                            psum.tile(
                                [VW, QSPAN], F32, tag="vot", bufs=BUFS_VOT,
                                name=f"ot_{pair}_{j}_{hh}",
                            )
                            for hh in range(2)
                        ]
                    ots = st["ots"]
                    o = max(kt_i * KT - qs, 0)  # window start (diag offset)
                    w = QSPAN - o
                    ps_s = psum.tile(
                        [128, 2 * QSPAN], F32, tag="s2", bufs=BUFS_S2,
                        name=f"pss_{pair}_{j}_{kt_i}",
                    )
                    for hh in range(2):
                        rowb = 64 * hh
                        nc.tensor.matmul(
                            ps_s[:, QSPAN * hh + o : QSPAN * hh + QSPAN],
                            rr(rotkh[
                                rowb : rowb + 64,
                                pair * S + kt_i * KT : pair * S + kt_i * KT + KT,
                            ]),
                            rr(rotqh[
                                rowb : rowb + 64,
                                pair * S + qs + o : pair * S + qs + QSPAN,
                            ]),
                            start=True,
                            stop=True,
                            tile_position=(rowb, 0),
                        )
                    e_t = epool.tile(
                        [128, 2 * QSPAN], MMDT, tag="e", name=f"e_{pair}_{j}_{kt_i}"
                    )
                    e3 = e_t.rearrange("p (t w) -> p t w", t=2)
                    s3 = ps_s.rearrange("p (t w) -> p t w", t=2)
                    if o == 0:
                        nc.scalar.activation(e_t[:], ps_s[:], EXP, scale=SCALE)
                    else:
                        nc.scalar.activation(
                            e3[:, :, o:QSPAN], s3[:, :, o:QSPAN], EXP, scale=SCALE
                        )
                    if kt_i * KT >= qs:
                        # diagonal block: zero strictly-above-diagonal. Only
                        # the first KT columns of the window intersect the
                        # diagonal; the rest are fully causal-valid.
                        if MASK_MUL:
                            nc.vector.tensor_mul(
                                e3[:, :, o : o + KT],
                                e3[:, :, o : o + KT],
                                cmask[:].unsqueeze(1).broadcast_to([128, 2, KT]),
                            )
                        else:
                            nc.gpsimd.affine_select(
                                out=e3[:, :, o:QSPAN],
                                in_=e3[:, :, o:QSPAN],
                                compare_op=mybir.AluOpType.is_ge,
                                fill=0.0,
                                base=0,
                                pattern=[[0, 2], [1, w]],
                                channel_multiplier=-1,
                            )
                    for hh in range(2):
                        h = 2 * pair + hh
                        nc.tensor.matmul(
                            ots[hh][:, o:QSPAN],
                            mr(vsb[
                                :,
                                h * (NSC * VW)
                                + kt_i * VW : h * (NSC * VW)
                                + kt_i * VW
                                + VW,
                            ]),
                            mr(e_t[:, QSPAN * hh + o : QSPAN * hh + QSPAN]),
                            start=(kt_i == 0),
                            stop=(kt_i == nkt - 1),
                        )

                def norm(hh):
                    ots = st["ots"]
                    # copy PSUM out first so the psum slot frees after one op
                    # instead of after the rec->bc->mul chain (gpsimd cannot
                    # read PSUM on hardware, so the chain must run from SBUF);
                    # split the two copies across DVE and ScalarE so the two
                    # heads' chains start in parallel
                    ost = rtmp.tile(
                        [VW, QSPAN], F32, tag="ost", bufs=2,
                        name=f"ost_{pair}_{j}_{hh}",
                    )
                    if hh == 0:
                        nc.vector.tensor_copy(ost[:], ots[hh][:])
                    else:
                        nc.scalar.copy(ost[:], ots[hh][:])
                    if NORM_CHUNKED:
                        # chunked normalize on gpsimd: first y-proj chunk can
                        # start early and DVE stays free
                        for ch in range(2):
                            cl = slice(256 * ch, 256 * ch + 256)
                            rec = rtmp.tile(
                                [1, 256], F32, tag="rec", bufs=4,
                                name=f"rec_{pair}_{j}_{hh}_{ch}",
                            )
                            bc = rtmp.tile(
                                [DK, 256], F32, tag="bc", bufs=4,
                                name=f"bc_{pair}_{j}_{hh}_{ch}",
                            )
                            nc.vector.reciprocal(rec[:], ost[DK : DK + 1, cl])
                            nc.gpsimd.partition_broadcast(bc[:], rec[0:1, :])
                            nc.gpsimd.tensor_mul(
                                attn_t[
                                    64 * hh : 64 * hh + 64,
                                    qs + 256 * ch : qs + 256 * ch + 256,
                                ],
                                ost[0:DK, cl],
                                bc[:],
                            )
                    else:
                        rec = rtmp.tile(
                            [1, QSPAN], F32, tag="rec", bufs=2,
                            name=f"rec_{pair}_{j}_{hh}",
                        )
                        bc = rtmp.tile(
                            [DK, QSPAN], F32, tag="bc", bufs=2,
                            name=f"bc_{pair}_{j}_{hh}",
                        )
                        nc.vector.reciprocal(rec[:], ost[DK : DK + 1, :])
                        nc.gpsimd.partition_broadcast(bc[:], rec[0:1, :])
                        nc.vector.tensor_mul(
                            attn_t[64 * hh : 64 * hh + 64, qs : qs + QSPAN],
                            ost[0:DK, :],
                            bc[:],
                        )

                units = [(lambda kt_i=kt_i: block(kt_i)) for kt_i in range(nkt)]
                units.append(lambda: norm(0))
                units.append(lambda: norm(1))
                return units

            def yproj_units(j):
                def sc_unit(scl):
                    sc = (QSPAN // KT) * j + scl
                    ysb = ypool.tile([128, D], DRDT, tag="ysb", name=f"ysb_{sc}")
                    for oh in range(2):
                        psy = psum.tile(
                            [128, 512], F32, tag="qk", bufs=BUFS_QK,
                            name=f"psy_{sc}_{oh}",
                        )
                        for p, attn_t in enumerate((attn0, attn1)):
                            nc.tensor.matmul(
                                psy[:],
                                mr(attn_t[:, KT * sc : KT * sc + KT]),
                                mr(wo_sb[:, D * p + 512 * oh : D * p + 512 * oh + 512]),
                                start=(p == 0),
                                stop=(p == 1),
                            )
                        # gpsimd cannot read PSUM on hardware: evict via DVE
                        nc.vector.tensor_copy(
                            ysb[:, 512 * oh : 512 * oh + 512], psy[:]
                        )
                    nc.sync.dma_start(y[KT * sc : KT * sc + KT, :], ysb[:])

                return [(lambda scl=scl: sc_unit(scl)) for scl in range(QSPAN // KT)]

            def interleave(base, fillers):
                """Spread fillers evenly among base units (after base[0])."""
                if not fillers:
                    return list(base)
                out = []
                n, m = len(base), len(fillers)
                step = max(1, n // (m + 1))
                fi = 0
                for bi, b in enumerate(base):
                    out.append(b)
                    if fi < m and (bi + 1) % step == 0 and bi < n - 1:
                        out.append(fillers[fi])
                        fi += 1
                out.extend(fillers[fi:])
                return out

            def _emit_once():
                for u in proj_units(0):
                    u()
                for u in proj_units(1):
                    u()
                for j in range(NQS):
                    pu = proj_units(j + 2) if j + 2 < NQS else None
                    ypf = yproj_units(j - 1) if j >= 1 else []
                    # v-proj units share the "vot" psum tag with the attention
                    # ot accumulators, so they may only run at pair boundaries
                    # (right after a pair's norm units release the slots)
                    qkf = [pu[0], pu[3]] if pu else []
                    vf = [pu[1], pu[2], pu[4], pu[5]] if pu else []
                    p0 = attn_pair_units(0, j)
                    p1 = attn_pair_units(1, j)
                    half = (len(qkf) + len(ypf) + 1) // 2
                    fill_all = qkf[:1] + ypf + qkf[1:]
                    seq = (
                        interleave(p0, fill_all[:half])
                        + vf[:2]
                        + interleave(p1, fill_all[half:])
                        + vf[2:]
                    )
                    for u in seq:
                        u()
                for u in yproj_units(NQS - 1):
                    u()

            # -------- interleaved emission: overlap exp/attention with proj --------
            for _rep in range(repeat):
                _emit_once()

    nc.compile()
    return nc


def get_program(repeat=1, k64=True):
    key = ("nc", repeat, k64)
    if key not in _PROGRAM_CACHE:
        _PROGRAM_CACHE[key] = _build_program(repeat, k64)
    return _PROGRAM_CACHE[key]


def make_core_inputs(x, token_positions, Wq, Wk, Wv, Wo):
    HDT = BF16 if globals().get("_MM_BF16", True) else np.float32
    """Build the 8 per-core input dicts (host-side sharding + layout prep)."""
    x = np.asarray(x, dtype=np.float32)
    pos = np.asarray(token_positions)
    Wq, Wk, Wv, Wo = (np.asarray(w, dtype=np.float32) for w in (Wq, Wk, Wv, Wo))

    inv_freq = 1.0 / (ROPE_THETA ** (np.arange(0, DK, 2, dtype=np.float32) / DK))
    ang = pos.astype(np.float32)[:, None] * inv_freq[None, :]  # [S, 32]
    cos32 = np.cos(ang).T.astype(HDT)  # [32, S]
    sin32 = np.sin(ang).T.astype(HDT)
    cosT = np.ascontiguousarray(np.tile(cos32, (4, 1)))  # [128, S]
    sinT = np.ascontiguousarray(np.tile(sin32, (4, 1)))

    in_maps = []
    for c in range(NCORES):
        b, g = c // 4, c % 4
        cols = np.array(
            [
                (4 * g + hl) * 64 + 2 * i + chunk
                for chunk in range(2)
                for hl in range(HPC)
                for i in range(32)
            ]
        )
        in_maps.append(
            {
                "xT": np.ascontiguousarray(x[b].T.astype(HDT)),
                "wqT": np.ascontiguousarray(Wq[cols, :].T.astype(HDT)),
                "wkT": np.ascontiguousarray(Wk[cols, :].T.astype(HDT)),
                "wvT": np.ascontiguousarray(Wv[256 * g : 256 * (g + 1), :].T.astype(HDT)),
                "woT": np.ascontiguousarray(Wo[:, 256 * g : 256 * (g + 1)].T.astype(HDT)),
                "cosT": cosT,
                "sinT": sinT,
            }
        )
    return in_maps


def kernel(x, token_positions, Wq, Wk, Wv, Wo, _trace=False):
    from concourse.bass_utils import run_bass_kernel_spmd

    nc = get_program()
    in_maps = make_core_inputs(x, token_positions, Wq, Wk, Wv, Wo)
    res = run_bass_kernel_spmd(
        nc, in_maps, core_ids=list(range(NCORES)), trace=_trace
    )
    out = np.zeros((B, S, D), dtype=np.float32)
    for c in range(NCORES):
        out[c // 4] += np.asarray(res.results[c]["y"]).astype(np.float32)
    if _trace:
        kernel.last_results = res
    return out

